# revision 3
# baseline (speedup 1.0000x reference)
"""Single-head attention (B=8, S=4096, E=512, H=64) on 8 trn2 NeuronCores.

Sharding: data-parallel over batch — one batch element per core.

Per-core algorithm (batch b):
  - Host pre-transposes x[b] -> xT [E, S] (layout prep only).
  - QKV: Q^T,K^T [H, S] head-major and V' [S, H+1] S-major (ones column
    appended) via PE matmuls over E-chunks; biases folded in as K=1
    rank-1 matmuls (bias x ones-row).
  - Scores computed TRANSPOSED: S^T[sk, sq] = K^T.T @ Q^T so softmax runs
    along partitions and attn @ V needs no transpose of attn.
  - Mask applied additively PRE-exp using the PE's free lhsT transpose:
    S^T += mask_chunk.T @ (-32768 * I); the int32 mask is DMA'd with an
    SWDGE dtype-cast straight to bf16 {0,1} (exact), so masking costs no
    vector-engine passes at all. exp(scale*(qk - 32768*m)) underflows to
    exactly 0 on masked lanes.
  - exp on ACT with no max-subtraction (|scaled scores| < ~10, safe).
  - Softmax denominator comes free from the ones column of V':
    outT = V'.T @ attn^T accumulates [H+1, sq] where row H is the row sum.
  - Tiny fixup: PE-transpose outT, reciprocal + scale on the [128, 64]
    output, DMA out.

Everything fp32 except: mask path (bf16, exact for {0,1} / -32768) and,
in "f32r" precision mode, the two big matmul groups (scores, attn@V) plus
QKV operand storage, which use float32r (~1.5e-4 matmul rel err, 4x faster
than fp32 on the PE).
"""
import sys

sys.path.insert(0, "/opt/trn_rl_repo")

import numpy as np

import concourse.bacc as bacc
import concourse.tile as tile
from concourse import mybir
from concourse.bass_utils import run_bass_kernel_spmd

F32 = mybir.dt.float32
F32R = mybir.dt.float32r
BF16 = mybir.dt.bfloat16
I32 = mybir.dt.int32

B, S, E, H = 8, 4096, 512, 64
SCALE = float(E) ** -0.5
NEG = -32768.0

PREC = "f32r"  # "f32" (exact) or "f32r" (fast PE mode for big matmuls)
MASK_CAST_DMA = False  # SWDGE int32->bf16 cast during DMA; else DVE convert


def build_program(s=S, prec=PREC, mask_cast=MASK_CAST_DMA):
    nc = bacc.Bacc("TRN2", target_bir_lowering=False, debug=False, num_devices=B)
    xT = nc.dram_tensor("xT", [E, s], F32, kind="ExternalInput")
    mask = nc.dram_tensor("mask", [s, s], I32, kind="ExternalInput")
    wq = nc.dram_tensor("wq", [E, H], F32, kind="ExternalInput")
    wk = nc.dram_tensor("wk", [E, H], F32, kind="ExternalInput")
    wv = nc.dram_tensor("wv", [E, H], F32, kind="ExternalInput")
    bq = nc.dram_tensor("bq", [1, H], F32, kind="ExternalInput")
    bk = nc.dram_tensor("bk", [1, H], F32, kind="ExternalInput")
    bv1 = nc.dram_tensor("bv1", [1, H + 1], F32, kind="ExternalInput")
    out = nc.dram_tensor("out", [s, H], F32, kind="ExternalOutput")

    DT = F32R if prec == "f32r" else F32
    NE = E // 128          # 4 E-chunks
    NB = s // 512          # q/s blocks of 512
    NQ = s // 128          # 128-row chunks

    with tile.TileContext(nc) as tc:
        with (
            tc.tile_pool(name="const", bufs=1) as cst,
            tc.tile_pool(name="xp", bufs=2) as xp,
            tc.tile_pool(name="qkv", bufs=1) as qkv,
            tc.tile_pool(name="maskp", bufs=6) as maskp,
            tc.tile_pool(name="maskip", bufs=3) as maskip,
            tc.tile_pool(name="atp", bufs=3) as atp,
            tc.tile_pool(name="osb", bufs=2) as osb,
        ):
            # ---- constants ----
            negI = cst.tile([128, 128], BF16)
            nc.gpsimd.memset(negI, 0.0)
            nc.gpsimd.affine_select(
                out=negI, in_=negI, compare_op=mybir.AluOpType.not_equal,
                fill=NEG, base=0, pattern=[[-1, 128]], channel_multiplier=1,
            )
            idf = cst.tile([128, 128], F32)
            nc.gpsimd.memset(idf, 0.0)
            nc.gpsimd.affine_select(
                out=idf, in_=idf, compare_op=mybir.AluOpType.not_equal,
                fill=1.0, base=0, pattern=[[-1, 128]], channel_multiplier=1,
            )
            ones512 = cst.tile([1, 512], F32)
            nc.vector.memset(ones512, 1.0)
            ones128 = cst.tile([1, 128], F32)
            nc.vector.memset(ones128, 1.0)

            wq_sb = cst.tile([128, NE, H], F32)
            wk_sb = cst.tile([128, NE, H], F32)
            wv_sb = cst.tile([128, NE, H], F32)
            for w_dram, w_sb in ((wq, wq_sb), (wk, wk_sb), (wv, wv_sb)):
                nc.sync.dma_start(
                    out=w_sb, in_=w_dram.rearrange("(c p) h -> p c h", p=128)
                )
            bq_sb = cst.tile([1, H], F32)
            bk_sb = cst.tile([1, H], F32)
            bv1_sb = cst.tile([1, H + 1], F32)
            nc.sync.dma_start(out=bq_sb, in_=bq[:])
            nc.sync.dma_start(out=bk_sb, in_=bk[:])
            nc.sync.dma_start(out=bv1_sb, in_=bv1[:])

            # ---- mask DMA (issue first: no deps, fills DMA queues early) ----
            mbs = []
            for qb in range(NB):
                row = []
                for j in range(4):
                    q0 = qb * 512 + j * 128
                    mb = maskp.tile([128, s], BF16, tag="mb", name=f"mb_{qb}_{j}")
                    if mask_cast:
                        nc.gpsimd.dma_start(out=mb, in_=mask[q0:q0 + 128, :])
                    else:
                        mi = maskip.tile([128, s], I32, tag="mi", name=f"mi_{qb}_{j}")
                        nc.scalar.dma_start(out=mi, in_=mask[q0:q0 + 128, :])
                        nc.vector.tensor_copy(mb, mi)
                    row.append(mb)
                mbs.append(row)

            # ---- phase A: QT, KT head-major; V' S-major ----
            QT = qkv.tile([H, s], DT)
            KT = qkv.tile([H, s], DT)
            VP = qkv.tile([128, NQ, H + 1], DT)
            with tc.tile_pool(name="psA", bufs=2, space="PSUM") as psA:
                for sb in range(NB):
                    s0 = sb * 512
                    xt = xp.tile([128, NE, 512], F32, tag="xt", name=f"xt_{sb}")
                    nc.sync.dma_start(
                        out=xt,
                        in_=xT[:, s0:s0 + 512].rearrange("(c p) s -> p c s", p=128),
                    )
                    for b_sb, w_sb, T in ((bq_sb, wq_sb, QT), (bk_sb, wk_sb, KT)):
                        t_ps = psA.tile([H, 512], F32, tag="qk", name=f"t_ps_{sb}")
                        nc.tensor.matmul(t_ps, b_sb, ones512, start=True, stop=False)
                        for e in range(NE):
                            nc.tensor.matmul(
                                t_ps, w_sb[:, e, :], xt[:, e, :],
                                start=False, stop=(e == NE - 1),
                            )
                        nc.vector.tensor_copy(T[:, s0:s0 + 512], t_ps)
                    for j in range(4):
                        k = sb * 4 + j
                        c0 = s0 + j * 128
                        v_ps = psA.tile([128, H + 1], F32, tag="v")
                        nc.tensor.matmul(v_ps, ones128, bv1_sb, start=True, stop=False)
                        for e in range(NE):
                            nc.tensor.matmul(
                                v_ps[:, 0:H], xt[:, e, c0 - s0:c0 - s0 + 128],
                                wv_sb[:, e, :], start=False, stop=(e == NE - 1),
                            )
                        nc.vector.tensor_copy(VP[:, k, :], v_ps)

            # ---- phase B: scores^T (+mask), exp, attn@V, fixup ----
            with (
                tc.tile_pool(name="psS", bufs=2, space="PSUM") as psS,
                tc.tile_pool(name="psO", bufs=2, space="PSUM") as psO,
                tc.tile_pool(name="psF", bufs=2, space="PSUM") as psF,
            ):
                for qb in range(NB):
                    q0 = qb * 512
                    ot_ps = psO.tile([H + 1, 512], F32, tag="ot")
                    for g in range(NQ // 2):
                        sc = psS.tile([128, 1024], F32, tag="sc")
                        for h2 in range(2):
                            k = 2 * g + h2
                            nc.tensor.matmul(
                                sc[:, 512 * h2:512 * h2 + 512],
                                KT[:, 128 * k:128 * (k + 1)],
                                QT[:, q0:q0 + 512],
                                start=True, stop=False,
                            )
                            for j in range(4):
                                c = 512 * h2 + 128 * j
                                nc.tensor.matmul(
                                    sc[:, c:c + 128],
                                    mbs[qb][j][:, 128 * k:128 * (k + 1)],
                                    negI, start=False, stop=(j == 3),
                                )
                        at = atp.tile([128, 1024], DT, tag="at")
                        nc.scalar.activation(
                            at, sc, mybir.ActivationFunctionType.Exp, scale=SCALE
                        )
                        for h2 in range(2):
                            k = 2 * g + h2
                            nc.tensor.matmul(
                                ot_ps, VP[:, k, :], at[:, 512 * h2:512 * h2 + 512],
                                start=(k == 0), stop=(k == NQ - 1),
                            )
                    oT = osb.tile([H + 1, 512], F32, tag="oT")
                    nc.vector.tensor_copy(oT, ot_ps)
                    for j in range(4):
                        fx = psF.tile([128, H + 1], F32, tag="fx")
                        nc.tensor.transpose(
                            fx, oT[:, 128 * j:128 * (j + 1)], idf[0:H + 1, 0:H + 1]
                        )
                        ob = osb.tile([128, H + 1], F32, tag="ob")
                        nc.vector.tensor_copy(ob, fx)
                        rc = osb.tile([128, 1], F32, tag="rc")
                        nc.vector.reciprocal(rc, ob[:, H:H + 1])
                        of = osb.tile([128, H], F32, tag="of")
                        nc.vector.tensor_scalar_mul(of, ob[:, 0:H], rc)
                        nc.sync.dma_start(
                            out=out[q0 + 128 * j:q0 + 128 * (j + 1), :], in_=of
                        )
    nc.compile()
    return nc


def make_in_maps(x, attention_mask, Wq, bq, Wk, bk, Wv, bv):
    nb = x.shape[0]
    bv1 = np.concatenate([bv, np.ones(1, np.float32)]).reshape(1, H + 1)
    common = {
        "wq": np.ascontiguousarray(Wq), "wk": np.ascontiguousarray(Wk),
        "wv": np.ascontiguousarray(Wv),
        "bq": np.ascontiguousarray(bq.reshape(1, H)),
        "bk": np.ascontiguousarray(bk.reshape(1, H)),
        "bv1": bv1,
    }
    return [
        {
            "xT": np.ascontiguousarray(x[b].T),
            "mask": np.ascontiguousarray(attention_mask[b]),
            **common,
        }
        for b in range(nb)
    ]


_PROGRAM = None


def kernel(x, attention_mask, Wq, bq, Wk, bk, Wv, bv):
    global _PROGRAM
    x = np.asarray(x, np.float32)
    attention_mask = np.asarray(attention_mask, np.int32)
    if _PROGRAM is None:
        _PROGRAM = build_program()
    in_maps = make_in_maps(
        x, attention_mask,
        np.asarray(Wq, np.float32), np.asarray(bq, np.float32),
        np.asarray(Wk, np.float32), np.asarray(bk, np.float32),
        np.asarray(Wv, np.float32), np.asarray(bv, np.float32),
    )
    res = run_bass_kernel_spmd(_PROGRAM, in_maps, core_ids=list(range(B)))
    return np.stack([res.results[b]["out"] for b in range(B)], axis=0)
